# revision 26
# baseline (speedup 1.0000x reference)
"""CameraAwareMemory loss kernel for 8 Trainium2 NeuronCores.

Strategy: shard the P=32768 proxy bank over 8 cores (4096 proxies each,
columns permuted camera-major).  All matmuls run as fp8e4m3 DoubleRow
(2 k-planes per instruction, K=256 in one matmul, half-rate columns):
  score = f_hi.m_hi + f_hi.m_lo + f_lo.m_hi   (3 DR matmuls, hi/lo fp8
          splitting gives ~bf16 dot accuracy)
  sims' = g8.m_hi                             (1 DR matmul; selection only)
Per (camera, row-tile) 512-proxy PSUM cell:
  - sims cells: DVE tensor_reduce(max, X) computes the max of every
    8-column group in one pass (PSUM -> SBUF f16 [128, 64]); the group
    maxes ship to the host, which takes top slots (a provable superset
    of any top-k of the raw sims) and recomputes exact scores at the
    ~8x-expanded candidate list.
  - score cells: scalar-engine exp(x - mhat) with per-camera accumulate
    (device-exact partial softmax denominators), except N_V8 cells per
    row-tile which instead return their top-8 scores (DVE max8) so the
    host can reconstruct those cells' exp-sums from the dominant terms
    (tail of a 512-cell beyond rank 8 is ~e^-16 of the total).
The host merges: logsumexp from the per-camera exp-sums, intra/cross
losses from exact positive scores, online loss from the exact top-k
recomputation at gm candidates.
"""

import sys

import numpy as np

sys.path.insert(0, "/opt/trn_rl_repo")

import ml_dtypes

F8 = ml_dtypes.float8_e4m3          # IEEE e4m3 (max normal 240) == TRN FP8_EXP4

# ---- problem constants (hardcoded per spec) ----
P = 32768
D = 256
C = 8
B = 256
TEMP = 0.05
BG_KNN = 50
POSK = 3
BAL_W = 0.15
RATIO = (1.0 - BAL_W) / BAL_W        # sims' = score + RATIO*q (same order as sims)
INV_TEMP = 1.0 / TEMP                # 20.0
NCORES = 8
PSH = P // NCORES                    # 4096 proxies per core
PCAM = PSH // C                      # 512 proxies per (core, camera)
G = 8                                # group size for the sims group-max
NSLOT = PCAM // G                    # 64 slots per (core, cam) cell

S_F = 32.0                           # fp8 scale for feat
S_M = 512.0                          # fp8 scale for memT
S_G = 32.0                           # fp8 scale for g = feat + RATIO*mem[prx]
ACT_SCALE = INV_TEMP / (S_F * S_M)   # psum -> x conversion for exp

# (rt, cam) pairs whose score cell uses the DVE max8 path instead of Act exp
V8_CELLS = [(0, 7)]
IN_PLACE_EXP = True                  # exp writes back onto the score PSUM tile
TABLE_PRELOAD = True                 # dummy Exp early to pay the table load
OUT_GPSIMD = False                   # route output DMAs through SWDGE (Pool)
SIMS_PAIR = 1                        # sims cells per DVE reduce op (1/2/4)
PS_Q_BUFS = 4
WARM_HEAVY = 0                       # wide warm-up DR matmuls (build the ramp)
WARM_HEAVY_W = 512
WARM_FEATHER = 0                     # feather warm-ups (burn queue slots)
WARM_FEATHER_W = 8

# input DMA chunking of the bank (columns of the 4096-wide shard; each chunk
# carries both k-planes and both hi/lo halves in one dma_start)
CHUNKS = [0, 512, 1024, 1536, 2048, 2560, 3072, 3584, 4096]
N_V8 = len(V8_CELLS)

_CACHE = {}


def _build_bass():
    import concourse.bacc as bacc
    import concourse.mybir as mybir
    import concourse.tile as tile
    from contextlib import ExitStack

    f32 = mybir.dt.float32
    f16 = mybir.dt.float16
    bf16 = mybir.dt.bfloat16
    f8 = mybir.dt.float8e4
    AF = mybir.ActivationFunctionType
    ALU = mybir.AluOpType
    DR = mybir.MatmulPerfMode.DoubleRow

    nc = bacc.Bacc("TRN2", target_bir_lowering=False, debug=False)

    # lhs: [128, 2, 6*128] fp8 DR-packed: (f_hi rt0|rt1, f_lo rt0|rt1, g8 rt0|rt1)
    lhs_d = nc.dram_tensor("lhs", [128, 2 * 768], f8, kind="ExternalInput")
    # bank: [128, 2, 8192] fp8 DR-packed: m_hi (4096 cols) | m_lo (4096 cols)
    bank_d = nc.dram_tensor("bank", [128, 2 * 8192], f8, kind="ExternalInput")
    nbias_d = nc.dram_tensor("nbias", [128, 2], f32, kind="ExternalInput")
    SVW = C + 8 * max(N_V8, 1)
    sv_d = nc.dram_tensor("sv", [B, SVW], f32, kind="ExternalOutput")
    gm_d = nc.dram_tensor("gm", [B, C * NSLOT], bf16, kind="ExternalOutput")

    with tile.TileContext(nc) as tc, ExitStack() as ctx:
        consts = ctx.enter_context(tc.tile_pool(name="consts", bufs=1))
        ps_s = ctx.enter_context(tc.tile_pool(name="ps_s", bufs=1, space="PSUM"))  # tags s0,s1 -> 2 pair tiles
        ps_q = ctx.enter_context(tc.tile_pool(name="ps_q", bufs=PS_Q_BUFS, space="PSUM"))
        psum_warm = ctx.enter_context(
            tc.tile_pool(name="psumw", bufs=1, space="PSUM"))
        small = ctx.enter_context(tc.tile_pool(name="small", bufs=2))
        outp = ctx.enter_context(tc.tile_pool(name="outp", bufs=2))

        lhs_sb = consts.tile([128, 2, 768], f8, tag="lhs")
        bank_sb = consts.tile([128, 2, 8192], f8, tag="bank")
        nc.sync.dma_start(
            out=lhs_sb[:], in_=lhs_d.rearrange("p (two c) -> p two c", two=2))
        # chunked bank DMA: each chunk covers both planes and both halves
        bank_src = bank_d.rearrange("p (two h c) -> p two h c", two=2, h=2)
        bank_dst = bank_sb.rearrange("p two (h c) -> p two h c", h=2)
        for g_ in range(len(CHUNKS) - 1):
            lo, hi = CHUNKS[g_], CHUNKS[g_ + 1]
            nc.sync.dma_start(
                out=bank_dst[:, :, :, lo:hi], in_=bank_src[:, :, :, lo:hi])

        mb = small.tile([128, 2], f32, tag="mbneg")
        nc.gpsimd.dma_start(out=mb[:], in_=nbias_d[:, :])
        mbneg = [mb[:, rt:rt + 1] for rt in range(2)]

        # PE warm-up on a memset tile. Heavy instructions build >3us of
        # continuous-execution ramp; feather instructions burn SEQ/queue
        # slots so every real matmul's cost locks after the ramp is hot.
        warm_in = consts.tile([128, 1024], f32, tag="warm")
        nc.gpsimd.memset(warm_in[:], 0.0)
        if TABLE_PRELOAD:
            # preload the Exp activation table while the bank DMA streams
            tl_t = small.tile([128, 8], f32, tag="tl")
            nc.scalar.activation(tl_t[:], warm_in[:, 0:8], AF.Exp,
                                 bias=0.0, scale=0.0)
        wl = warm_in[:, 0:64].bitcast(f8).rearrange("p (two m) -> p two m", two=2)
        warm_ps = psum_warm.tile([128, max(WARM_HEAVY_W, WARM_FEATHER_W)], f32,
                                 tag="warmps")
        wrh = warm_in[:, 64:64 + WARM_HEAVY_W // 2].bitcast(f8).rearrange(
            "p (two x) -> p two x", two=2)
        wrf = warm_in[:, 64:64 + WARM_FEATHER_W // 2].bitcast(f8).rearrange(
            "p (two x) -> p two x", two=2)
        for _ in range(WARM_HEAVY):
            nc.tensor.matmul(warm_ps[:, 0:WARM_HEAVY_W], lhsT=wl, rhs=wrh,
                             start=True, stop=True, perf_mode=DR)
        for _ in range(WARM_FEATHER):
            nc.tensor.matmul(warm_ps[:, 0:WARM_FEATHER_W], lhsT=wl, rhs=wrf,
                             start=True, stop=True, perf_mode=DR)

        def lhs_ap(which, rt):
            # which: 0=f_hi, 1=f_lo, 2=g8
            base = which * 256 + rt * 128
            return lhs_sb[:, :, base:base + 128]

        m_hi = bank_sb[:, :, 0:4096]
        m_lo = bank_sb[:, :, 4096:8192]

        gm_t, sv_t = [], []
        for rt in range(2):
            gm_t.append(outp.tile([128, C * NSLOT], bf16, tag=f"gm{rt}",
                                  name=f"gm_{rt}"))
            sv_t.append(outp.tile([128, SVW], f32, tag=f"sv{rt}",
                                  name=f"sv_{rt}"))
        scam_t = [sv_t[rt][:, 0:C] for rt in range(2)]
        v8_t = [sv_t[rt][:, C:] for rt in range(2)]
        e_t = [outp.tile([128, 512], f32, tag=f"e{rt}", name=f"e_{rt}")
               for rt in range(2)]
        _pair = {}

        # per camera: sims matmul (grouped SIMS_PAIR cells per reduce) +
        # score trio; both row-tiles inner so chunk deps stay monotone.
        SP_ = SIMS_PAIR
        for cam in range(C):
            c0, c1 = cam * 512, (cam + 1) * 512
            for rt in range(2):
                if cam % SP_ == 0:
                    pss = ps_s.tile([128, 512 * SP_], f32, tag=f"s{rt}",
                                    name=f"pss_{cam // SP_}_{rt}")
                    _pair[rt] = pss
                else:
                    pss = _pair[rt]
                nc.tensor.matmul(pss[:, (cam % SP_) * 512:(cam % SP_ + 1) * 512],
                                 lhsT=lhs_ap(2, rt), rhs=m_hi[:, :, c0:c1],
                                 start=True, stop=True, perf_mode=DR)
                if cam % SP_ == SP_ - 1:
                    nc.vector.tensor_reduce(
                        out=gm_t[rt][:, (cam + 1 - SP_) * NSLOT:(cam + 1) * NSLOT],
                        in_=pss.rearrange("p (q s g) -> p q s g", q=SP_, g=G),
                        axis=mybir.AxisListType.X, op=ALU.max)
                psq = ps_q.tile([128, 512], f32, tag="q",
                                name=f"psq_{cam}_{rt}")
                nc.tensor.matmul(psq[:], lhsT=lhs_ap(0, rt),
                                 rhs=m_hi[:, :, c0:c1],
                                 start=True, stop=False, perf_mode=DR)
                nc.tensor.matmul(psq[:], lhsT=lhs_ap(0, rt),
                                 rhs=m_lo[:, :, c0:c1],
                                 start=False, stop=False, perf_mode=DR)
                nc.tensor.matmul(psq[:], lhsT=lhs_ap(1, rt),
                                 rhs=m_hi[:, :, c0:c1],
                                 start=False, stop=True, perf_mode=DR)
                if (rt, cam) in V8_CELLS:
                    j = V8_CELLS.index((rt, cam))
                    nc.vector.max(v8_t[rt][:, j * 8:(j + 1) * 8], psq[:])
                else:
                    eo = psq[:] if IN_PLACE_EXP else e_t[rt][:]
                    nc.scalar.activation(
                        eo, psq[:], AF.Exp,
                        bias=mbneg[rt][:], scale=ACT_SCALE,
                        accum_out=scam_t[rt][:, cam:cam + 1],
                    )
            if cam == 3:
                nf = 4 * NSLOT
                for rt in range(2):
                    r0, r1 = rt * 128, (rt + 1) * 128
                    dq = nc.gpsimd if OUT_GPSIMD else nc.sync
                    dq.dma_start(out=gm_d[r0:r1, 0:nf], in_=gm_t[rt][:, 0:nf])

        nf = 4 * NSLOT
        for rt in range(2):
            r0, r1 = rt * 128, (rt + 1) * 128
            dq = nc.gpsimd if OUT_GPSIMD else nc.sync
            dq.dma_start(out=gm_d[r0:r1, nf:], in_=gm_t[rt][:, nf:])
            dq.dma_start(out=sv_d[r0:r1, :], in_=sv_t[rt][:])

    nc.compile()
    return nc


def _get_nc():
    if "nc" not in _CACHE:
        _CACHE["nc"] = _build_bass()
    return _CACHE["nc"]


def _run_device(in_maps, trace=False):
    from concourse.bass_utils import run_bass_kernel_spmd

    nc = _get_nc()
    res = run_bass_kernel_spmd(
        nc, in_maps, core_ids=list(range(NCORES)), trace=trace
    )
    return res


def _q8(x, clip=240.0):
    return np.clip(x, -clip, clip).astype(F8)


def _drpack(mat):
    """[256, X] (k-major) -> [128, 2, X]: plane j holds k = j*128 + p."""
    return np.stack([mat[0:128], mat[128:256]], axis=1)


def kernel(features, targets, cams, epoch, global_memory, all_pseudo_label,
           all_proxy_label, cam_proxies, label_proxies, _want_trace=False):
    feat = np.ascontiguousarray(np.asarray(features), dtype=np.float32)
    mem = np.ascontiguousarray(np.asarray(global_memory), dtype=np.float32)
    targets = np.asarray(targets).astype(np.int64)
    cams_h = np.asarray(cams).astype(np.int64)
    apl = np.asarray(all_proxy_label).astype(np.int64)
    apsl = np.asarray(all_pseudo_label).astype(np.int64)
    cam_prox = np.asarray(cam_proxies).astype(np.int64)
    lab_prox = np.asarray(label_proxies).astype(np.int64)

    prx = apl[targets]                      # [B] target proxy
    pseudo_y = apsl[targets]                # [B]
    pos_cols = lab_prox[pseudo_y]           # [B, K] positive proxies (cross)
    memprx = mem[prx]                       # [B, D]
    g = feat + np.float32(RATIO) * memprx   # [B, D]

    # camera of each proxy, and the camera-major permutation per core shard
    cam_of_p = np.empty(P, np.int64)
    cam_of_p[cam_prox.reshape(-1)] = np.repeat(np.arange(C), cam_prox.shape[1])
    perms = np.empty((NCORES, PSH), np.int64)
    for k in range(NCORES):
        ids = np.arange(k * PSH, (k + 1) * PSH)
        parts = [ids[cam_of_p[ids] == c] for c in range(C)]
        assert all(len(p) == PCAM for p in parts), "camera layout mismatch"
        perms[k] = np.concatenate(parts)

    # fp8 hi/lo quantization (shared across cores; shards sliced per core)
    featT = feat.T                                   # [D, B]
    f_hi8 = _q8(featT * np.float32(S_F))
    f_lo8 = _q8(featT * np.float32(S_F) - f_hi8.astype(np.float32))
    g_hi8 = _q8(g.T * np.float32(S_G))
    memT = mem.T                                     # [D, P]
    assert np.abs(featT).max() * S_F < 239, "feat fp8 scale overflow"
    assert np.abs(g).max() * S_G < 239, "g fp8 scale overflow"
    assert np.abs(memT).max() * S_M < 239, "mem fp8 scale overflow"

    # Row-global exp bias (identical across cores so the merge is a plain sum)
    mhat = (4.5 * INV_TEMP / np.sqrt(D)) * np.linalg.norm(
        feat.astype(np.float64), axis=1)    # [B]
    nbias = np.ascontiguousarray(
        (-mhat.reshape(2, 128).T).astype(np.float32))      # [128, 2] col=rt

    in_maps = []
    for k in range(NCORES):
        mT = memT[:, perms[k]] * np.float32(S_M)     # [D, 4096]
        m_hi8 = _q8(mT)
        m_lo8 = _q8(mT - m_hi8.astype(np.float32))
        lhs = np.concatenate([
            _drpack(f_hi8[:, 0:128]), _drpack(f_hi8[:, 128:256]),
            _drpack(f_lo8[:, 0:128]), _drpack(f_lo8[:, 128:256]),
            _drpack(g_hi8[:, 0:128]), _drpack(g_hi8[:, 128:256]),
        ], axis=2)                                   # [128, 2, 768]
        bank = np.concatenate([_drpack(m_hi8), _drpack(m_lo8)], axis=2)
        in_maps.append({
            "lhs": np.ascontiguousarray(lhs.reshape(128, -1)),
            "bank": np.ascontiguousarray(bank.reshape(128, -1)),
            "nbias": nbias,
        })

    res = _run_device(in_maps, trace=_want_trace)
    results = res.results
    if _want_trace:
        _CACHE["last_exec_time_ns"] = res.exec_time_ns

    sv = np.stack([r["sv"] for r in results]).astype(np.float64)      # [K,B,SVW]
    gm = np.stack([r["gm"] for r in results]).astype(np.float32)      # [K,B,C*64]
    scam = sv[:, :, 0:C]
    v8 = sv[:, :, C:]

    rows = np.arange(B)
    mhat_used = -nbias.T.reshape(B).astype(np.float64)  # [B] f32-exact

    # ---- per-(core,cam) exp-sum partials ----
    # Act cells: device-exact. v8 cells: top-8 reconstruction (tail ~ e^-16).
    parts = scam.copy()                                # [K, B, C]
    for j, (rt, cam) in enumerate(V8_CELLS):
        r0, r1 = rt * 128, (rt + 1) * 128
        x8 = v8[:, r0:r1, j * 8:(j + 1) * 8] * ACT_SCALE   # [K, 128, 8]
        parts[:, r0:r1, cam] = np.exp(
            x8 - mhat_used[None, r0:r1, None]).sum(axis=2)
    Zc = parts.sum(axis=0)                             # [B, C]
    lse_full = mhat_used + np.log(Zc.sum(axis=1))
    lse_cam = mhat_used + np.log(Zc[rows, cams_h])

    x_prx = INV_TEMP * np.einsum("bd,bd->b", feat.astype(np.float64),
                                 memprx.astype(np.float64))
    present = cam_of_p[prx] == cams_h
    intra = np.where(present, lse_cam - x_prx, 0.0)

    x_pos = INV_TEMP * np.einsum("bd,bkd->bk", feat.astype(np.float64),
                                 mem[pos_cols].astype(np.float64))
    cross = lse_full - x_pos.mean(axis=1)

    # ---- online loss from gm slots ----
    # slot (k, cam, s) covers proxies perm[k][cam*512 + s*8 + (0..7)]
    V = gm.reshape(NCORES, B, C, NSLOT)                # [K, B, C, S]
    Vb = np.moveaxis(V, 1, 0).reshape(B, NCORES * C * NSLOT)  # [B, K*C*S]
    slot_cam = np.tile(np.repeat(np.arange(C), NSLOT), NCORES)
    slot_core = np.repeat(np.arange(NCORES), C * NSLOT)
    slot_off = np.tile(np.arange(NSLOT), NCORES * C)

    TGLOB = 96
    top_glob = np.argpartition(-Vb, TGLOB, axis=1)[:, :TGLOB]  # [B, 96]
    # per-camera top-4 slots (argmax coverage with fp16 slack)
    percam = []
    for c in range(C):
        cols = np.where(slot_cam == c)[0]
        sub = Vb[:, cols]
        a = np.argpartition(-sub, 4, axis=1)[:, :4]
        percam.append(cols[a])
    slots_sel = np.concatenate([top_glob] + percam, axis=1)    # [B, 96+32]
    nsel = slots_sel.shape[1]

    # candidate proxies: [B, nsel, G]
    base = (slot_core[slots_sel] * PSH + slot_cam[slots_sel] * PCAM
            + slot_off[slots_sel] * G)                         # [B, nsel]
    pid = perms.reshape(-1)[
        (slot_core[slots_sel] * PSH)[..., None]
        + (slot_cam[slots_sel] * PCAM + slot_off[slots_sel] * G)[..., None]
        + np.arange(G)[None, None, :]]                         # [B, nsel, G]
    pid_b = pid.reshape(B, nsel * G)
    cam_of_cand = np.repeat(slot_cam[slots_sel], G, axis=1)    # [B, nsel*G]

    memg = mem[pid_b]                                          # [B, J, D]
    s_cand = np.einsum("bd,bjd->bj", feat, memg)               # exact f32 score
    q_cand = np.einsum("bd,bjd->bj", memprx, memg)
    simsp = s_cand.astype(np.float64) + RATIO * q_cand.astype(np.float64)
    x_cand = INV_TEMP * s_cand.astype(np.float64)

    # dedupe duplicate proxies (same slot can appear in top_glob and percam)
    order_j = np.argsort(pid_b, axis=1)
    pid_srt = np.take_along_axis(pid_b, order_j, axis=1)
    dup = np.zeros_like(pid_b, dtype=bool)
    dup_srt = np.zeros_like(dup)
    dup_srt[:, 1:] = pid_srt[:, 1:] == pid_srt[:, :-1]
    np.put_along_axis(dup, order_j, dup_srt, axis=1)
    simsp = np.where(dup, -np.inf, simsp)

    # per-camera global argmax over candidates (exact values)
    tops_val = np.full((B, C), -np.inf)
    tops_j = np.zeros((B, C), np.int64)
    for c in range(C):
        m = cam_of_cand == c
        sub = np.where(m, simsp, -np.inf)
        a = sub.argmax(axis=1)
        tops_j[:, c] = a
        tops_val[:, c] = sub[rows, a]

    order = np.argsort(-tops_val, axis=1)[:, :POSK]            # [B, 3]
    chosen_j = np.take_along_axis(tops_j, order, axis=1)       # [B, 3]
    chosen_pid = np.take_along_axis(pid_b, chosen_j, axis=1)

    is_chosen = (pid_b[:, :, None] == chosen_pid[:, None, :]).any(axis=2)
    Vmask = np.where(is_chosen, -np.inf, simsp)
    sel_idx = np.argpartition(-Vmask, BG_KNN, axis=1)[:, :BG_KNN]

    x_chosen = np.take_along_axis(x_cand, chosen_j, axis=1)    # [B, 3]
    x_sel = np.take_along_axis(x_cand, sel_idx, axis=1)        # [B, 50]
    xA = np.concatenate([x_chosen, x_sel], axis=1)             # [B, 53]
    mA = xA.max(axis=1)
    lse3 = mA + np.log(np.exp(xA - mA[:, None]).sum(axis=1))
    online = lse3 - x_chosen.mean(axis=1)

    dbg = globals().get("_DEBUG_COMPS")
    if dbg is not None:
        dbg["intra"] = intra.copy()
        dbg["cross"] = cross.copy()
        dbg["online"] = online.copy()
    total = 0.0
    for c in range(C):
        m = cams_h == c
        if m.any():
            total += intra[m].mean() + cross[m].mean() + online[m].mean()
    return np.float32(total)


# revision 28
# speedup vs baseline: 1.0232x; 1.0232x over previous
"""CameraAwareMemory loss kernel for 8 Trainium2 NeuronCores.

Strategy: shard the P=32768 proxy bank over 8 cores (4096 proxies each,
columns permuted camera-major).  All matmuls run as fp8e4m3 DoubleRow
(2 k-planes per instruction, K=256 in one matmul, half-rate columns):
  score = f_hi.m_hi + f_hi.m_lo + f_lo.m_hi   (3 DR matmuls, hi/lo fp8
          splitting gives ~bf16 dot accuracy)
  sims' = g8.m_hi                             (1 DR matmul; selection only)
Per (camera, row-tile) 512-proxy PSUM cell:
  - sims cells: DVE tensor_reduce(max, X) computes the max of every
    8-column group in one pass (PSUM -> SBUF f16 [128, 64]); the group
    maxes ship to the host, which takes top slots (a provable superset
    of any top-k of the raw sims) and recomputes exact scores at the
    ~8x-expanded candidate list.
  - score cells: scalar-engine exp(x - mhat) with per-camera accumulate
    (device-exact partial softmax denominators), except N_V8 cells per
    row-tile which instead return their top-8 scores (DVE max8) so the
    host can reconstruct those cells' exp-sums from the dominant terms
    (tail of a 512-cell beyond rank 8 is ~e^-16 of the total).
The host merges: logsumexp from the per-camera exp-sums, intra/cross
losses from exact positive scores, online loss from the exact top-k
recomputation at gm candidates.
"""

import sys

import numpy as np

sys.path.insert(0, "/opt/trn_rl_repo")

import ml_dtypes

F8 = ml_dtypes.float8_e4m3          # IEEE e4m3 (max normal 240) == TRN FP8_EXP4

# ---- problem constants (hardcoded per spec) ----
P = 32768
D = 256
C = 8
B = 256
TEMP = 0.05
BG_KNN = 50
POSK = 3
BAL_W = 0.15
RATIO = (1.0 - BAL_W) / BAL_W        # sims' = score + RATIO*q (same order as sims)
INV_TEMP = 1.0 / TEMP                # 20.0
NCORES = 8
PSH = P // NCORES                    # 4096 proxies per core
PCAM = PSH // C                      # 512 proxies per (core, camera)
G = 8                                # group size for the sims group-max
NSLOT = PCAM // G                    # 64 slots per (core, cam) cell

S_F = 32.0                           # fp8 scale for feat
S_M = 512.0                          # fp8 scale for memT
S_G = 32.0                           # fp8 scale for g = feat + RATIO*mem[prx]
ACT_SCALE = INV_TEMP / (S_F * S_M)   # psum -> x conversion for exp

TABLE_PRELOAD = True                 # dummy Exp early to pay the table load
LHS_QUEUE = "sync"                   # which engine queue issues the lhs DMA
SIMS_PAIR = 1                        # sims cells per DVE reduce op (1/2)
M_SHIFT = 112.5                      # global exp shift: > max_b 4.5*20*|feat_b|/16

# input DMA chunking of the bank (columns of the 4096-wide shard; each chunk
# carries both k-planes and both hi/lo halves in one dma_start)
CHUNKS = [0, 512, 1024, 1536, 2048, 2560, 3072, 3584, 4096]

_CACHE = {}


def _build_bass():
    import concourse.bacc as bacc
    import concourse.mybir as mybir
    import concourse.tile as tile
    from contextlib import ExitStack

    f32 = mybir.dt.float32
    f32r = mybir.dt.float32r
    bf16 = mybir.dt.bfloat16
    f8 = mybir.dt.float8e4
    AF = mybir.ActivationFunctionType
    ALU = mybir.AluOpType
    DR = mybir.MatmulPerfMode.DoubleRow

    nc = bacc.Bacc("TRN2", target_bir_lowering=False, debug=False)

    # lhs: [128, 2, 6*128] fp8 DR-packed: (f_hi rt0|rt1, f_lo rt0|rt1, g8 rt0|rt1)
    lhs_d = nc.dram_tensor("lhs", [128, 2 * 768], f8, kind="ExternalInput")
    # bank: [128, 2, 8192] fp8 DR-packed: m_hi (4096 cols) | m_lo (4096 cols)
    bank_d = nc.dram_tensor("bank", [128, 2 * 8192], f8, kind="ExternalInput")
    gm_d = nc.dram_tensor("gm", [B, C * NSLOT + C], bf16, kind="ExternalOutput")

    with tile.TileContext(nc) as tc, ExitStack() as ctx:
        consts = ctx.enter_context(tc.tile_pool(name="consts", bufs=1))
        ps_cam = ctx.enter_context(tc.tile_pool(name="ps_cam", bufs=2, space="PSUM"))
        ps_s = ctx.enter_context(tc.tile_pool(name="ps_s", bufs=1, space="PSUM"))
        ps_sum = ctx.enter_context(tc.tile_pool(name="ps_sum", bufs=1, space="PSUM"))
        small = ctx.enter_context(tc.tile_pool(name="small", bufs=2))
        epool = ctx.enter_context(tc.tile_pool(name="ep", bufs=3))
        outp = ctx.enter_context(tc.tile_pool(name="outp", bufs=2))

        lhs_sb = consts.tile([128, 2, 768], f8, tag="lhs")
        bank_sb = consts.tile([128, 2, 8192], f8, tag="bank")
        lhs_src = lhs_d.rearrange("p (two c) -> p two c", two=2)
        bank_src = bank_d.rearrange("p (two h c) -> p two h c", two=2, h=2)
        bank_dst = bank_sb.rearrange("p two (h c) -> p two h c", h=2)
        # g8 block first (feeds the sims pipeline), then the first bank chunk,
        # then the f blocks (score), then the remaining chunks
        nc.sync.dma_start(out=lhs_sb[:, :, 512:768], in_=lhs_src[:, :, 512:768])
        lo, hi = CHUNKS[0], CHUNKS[1]
        nc.sync.dma_start(
            out=bank_dst[:, :, :, lo:hi], in_=bank_src[:, :, :, lo:hi])
        nc.sync.dma_start(out=lhs_sb[:, :, 0:512], in_=lhs_src[:, :, 0:512])
        for g_ in range(1, len(CHUNKS) - 1):
            lo, hi = CHUNKS[g_], CHUNKS[g_ + 1]
            nc.sync.dma_start(
                out=bank_dst[:, :, :, lo:hi], in_=bank_src[:, :, :, lo:hi])

        # ones column for the partition-sum matmuls (f32r reads the f32 bits)
        ones_t = consts.tile([128, 1], bf16, tag="ones")
        nc.gpsimd.memset(ones_t[:], 1.0)
        mneg_t = consts.tile([128, 1], f32, tag="mneg")
        nc.gpsimd.memset(mneg_t[:], -float(M_SHIFT))
        if TABLE_PRELOAD:
            jnk = small.tile([128, 8], f32, tag="jnk")
            nc.gpsimd.memset(jnk[:], 0.0)
            tl_t = small.tile([128, 8], f32, tag="tl")
            nc.scalar.activation(tl_t[:], jnk[:], AF.Exp, bias=0.0, scale=0.0)

        # rhs views for the transposed score matmuls: all-256-batch fp8 blocks
        f_hi_all = lhs_sb[:, :, 0:256]
        f_lo_all = lhs_sb[:, :, 256:512]

        def g8_ap(rt):
            return lhs_sb[:, :, 512 + rt * 128:512 + (rt + 1) * 128]

        m_hi = bank_sb[:, :, 0:4096]
        m_lo = bank_sb[:, :, 4096:8192]

        gm_t = []
        for rt in range(2):
            gm_t.append(outp.tile([128, C * NSLOT + C], bf16, tag=f"gm{rt}",
                                  name=f"gm_{rt}"))
        _pair = {}
        st_all = ps_sum.tile([128, 2 * C], f32, tag="st", name="st_all")
        st = [st_all[:, ch * C:(ch + 1) * C] for ch in range(2)]

        ecam_of = {}

        def emit_minis(cam):
            ecam = ecam_of.pop(cam)
            for ch in range(2):
                for kb in range(4):
                    nc.tensor.matmul(
                        st[ch][:, cam:cam + 1],
                        lhsT=ecam[:, kb * 256 + ch * 128:
                                  kb * 256 + ch * 128 + 128],
                        rhs=ones_t[:],
                        start=(kb == 0), stop=(kb == 3))

        for cam in range(C):
            c0 = cam * 512
            # --- sims (normal orientation): per row-tile matmul + group-max
            for rt in range(2):
                if cam % SIMS_PAIR == 0 or (rt not in _pair):
                    pss = ps_s.tile([128, 512 * SIMS_PAIR], f32, tag=f"s{rt}",
                                    name=f"pss_{cam // SIMS_PAIR}_{rt}")
                    _pair[rt] = pss
                pss = _pair[rt]
                sl = (cam % SIMS_PAIR)
                nc.tensor.matmul(pss[:, sl * 512:(sl + 1) * 512],
                                 lhsT=g8_ap(rt), rhs=m_hi[:, :, c0:c0 + 512],
                                 start=True, stop=True, perf_mode=DR)
                if sl == SIMS_PAIR - 1:
                    nc.vector.tensor_reduce(
                        out=gm_t[rt][:, (cam + 1 - SIMS_PAIR) * NSLOT:
                                     (cam + 1) * NSLOT],
                        in_=pss.rearrange("p (q s g) -> p q s g",
                                          q=SIMS_PAIR, g=G),
                        axis=mybir.AxisListType.X, op=ALU.max)

            # --- transposed score: 4 k-blocks x 3 DR matmuls -> [128p, 1024b]
            camq = ps_cam.tile([128, 1024], f32, tag="cam", name=f"camq_{cam}")
            for kb in range(4):
                mh = m_hi[:, :, c0 + kb * 128:c0 + (kb + 1) * 128]
                ml = m_lo[:, :, c0 + kb * 128:c0 + (kb + 1) * 128]
                cols = slice(kb * 256, (kb + 1) * 256)
                nc.tensor.matmul(camq[:, cols], lhsT=mh, rhs=f_hi_all,
                                 start=True, stop=False, perf_mode=DR)
                nc.tensor.matmul(camq[:, cols], lhsT=ml, rhs=f_hi_all,
                                 start=False, stop=False, perf_mode=DR)
                nc.tensor.matmul(camq[:, cols], lhsT=mh, rhs=f_lo_all,
                                 start=False, stop=True, perf_mode=DR)
            # exp of the whole camera (global shift; f32 out for range)
            ecam = epool.tile([128, 1024], bf16, tag="e", name=f"e_{cam}")
            nc.scalar.activation(ecam[:], camq[:], AF.Exp,
                                 bias=mneg_t[:], scale=ACT_SCALE)
            ecam_of[cam] = ecam
            # software pipeline: sums for the PREVIOUS camera (its exp has had
            # a full camera's worth of PE work to complete)
            if cam >= 1:
                emit_minis(cam - 1)
            if cam == C - 1:
                emit_minis(cam)
                for ch in range(2):
                    nc.scalar.activation(
                        gm_t[ch][:, C * NSLOT:], st[ch][:], AF.Copy,
                        bias=0.0, scale=1.0)
            if cam in (3, 5):
                lo = 0 if cam == 3 else 4 * NSLOT
                hi = (cam + 1) * NSLOT
                for rt in range(2):
                    r0, r1 = rt * 128, (rt + 1) * 128
                    nc.sync.dma_start(out=gm_d[r0:r1, lo:hi],
                                      in_=gm_t[rt][:, lo:hi])

        nf = 6 * NSLOT
        for rt in range(2):
            r0, r1 = rt * 128, (rt + 1) * 128
            nc.sync.dma_start(out=gm_d[r0:r1, nf:], in_=gm_t[rt][:, nf:])

    nc.compile()
    return nc


def _get_nc():
    if "nc" not in _CACHE:
        _CACHE["nc"] = _build_bass()
    return _CACHE["nc"]


def _run_device(in_maps, trace=False):
    from concourse.bass_utils import run_bass_kernel_spmd

    nc = _get_nc()
    res = run_bass_kernel_spmd(
        nc, in_maps, core_ids=list(range(NCORES)), trace=trace
    )
    return res


def _q8(x, clip=240.0):
    return np.clip(x, -clip, clip).astype(F8)


def _drpack(mat):
    """[256, X] (k-major) -> [128, 2, X]: plane j holds k = j*128 + p."""
    return np.stack([mat[0:128], mat[128:256]], axis=1)


def kernel(features, targets, cams, epoch, global_memory, all_pseudo_label,
           all_proxy_label, cam_proxies, label_proxies, _want_trace=False):
    feat = np.ascontiguousarray(np.asarray(features), dtype=np.float32)
    mem = np.ascontiguousarray(np.asarray(global_memory), dtype=np.float32)
    targets = np.asarray(targets).astype(np.int64)
    cams_h = np.asarray(cams).astype(np.int64)
    apl = np.asarray(all_proxy_label).astype(np.int64)
    apsl = np.asarray(all_pseudo_label).astype(np.int64)
    cam_prox = np.asarray(cam_proxies).astype(np.int64)
    lab_prox = np.asarray(label_proxies).astype(np.int64)

    prx = apl[targets]                      # [B] target proxy
    pseudo_y = apsl[targets]                # [B]
    pos_cols = lab_prox[pseudo_y]           # [B, K] positive proxies (cross)
    memprx = mem[prx]                       # [B, D]
    g = feat + np.float32(RATIO) * memprx   # [B, D]

    # camera of each proxy, and the camera-major permutation per core shard
    cam_of_p = np.empty(P, np.int64)
    cam_of_p[cam_prox.reshape(-1)] = np.repeat(np.arange(C), cam_prox.shape[1])
    perms = np.empty((NCORES, PSH), np.int64)
    for k in range(NCORES):
        ids = np.arange(k * PSH, (k + 1) * PSH)
        parts = [ids[cam_of_p[ids] == c] for c in range(C)]
        assert all(len(p) == PCAM for p in parts), "camera layout mismatch"
        perms[k] = np.concatenate(parts)

    # fp8 hi/lo quantization (shared across cores; shards sliced per core)
    featT = feat.T                                   # [D, B]
    f_hi8 = _q8(featT * np.float32(S_F))
    f_lo8 = _q8(featT * np.float32(S_F) - f_hi8.astype(np.float32))
    g_hi8 = _q8(g.T * np.float32(S_G))
    memT = mem.T                                     # [D, P]
    assert np.abs(featT).max() * S_F < 239, "feat fp8 scale overflow"
    assert np.abs(g).max() * S_G < 239, "g fp8 scale overflow"
    assert np.abs(memT).max() * S_M < 239, "mem fp8 scale overflow"

    in_maps = []
    for k in range(NCORES):
        mT = memT[:, perms[k]] * np.float32(S_M)     # [D, 4096]
        m_hi8 = _q8(mT)
        m_lo8 = _q8(mT - m_hi8.astype(np.float32))
        lhs = np.concatenate([
            _drpack(f_hi8[:, 0:128]), _drpack(f_hi8[:, 128:256]),
            _drpack(f_lo8[:, 0:128]), _drpack(f_lo8[:, 128:256]),
            _drpack(g_hi8[:, 0:128]), _drpack(g_hi8[:, 128:256]),
        ], axis=2)                                   # [128, 2, 768]
        bank = np.concatenate([_drpack(m_hi8), _drpack(m_lo8)], axis=2)
        in_maps.append({
            "lhs": np.ascontiguousarray(lhs.reshape(128, -1)),
            "bank": np.ascontiguousarray(bank.reshape(128, -1)),
        })

    res = _run_device(in_maps, trace=_want_trace)
    results = res.results
    if _want_trace:
        _CACHE["last_exec_time_ns"] = res.exec_time_ns

    gmall = np.stack([r["gm"] for r in results]).astype(np.float32)   # [K,B,C*64+C]
    gm = gmall[:, :, 0:C * NSLOT]
    parts = gmall[:, :, C * NSLOT:].astype(np.float64)                 # [K,B,C]

    rows = np.arange(B)
    Zc = parts.sum(axis=0)                             # [B, C]
    lse_full = M_SHIFT + np.log(Zc.sum(axis=1))
    lse_cam = M_SHIFT + np.log(Zc[rows, cams_h])

    x_prx = INV_TEMP * np.einsum("bd,bd->b", feat.astype(np.float64),
                                 memprx.astype(np.float64))
    present = cam_of_p[prx] == cams_h
    intra = np.where(present, lse_cam - x_prx, 0.0)

    x_pos = INV_TEMP * np.einsum("bd,bkd->bk", feat.astype(np.float64),
                                 mem[pos_cols].astype(np.float64))
    cross = lse_full - x_pos.mean(axis=1)

    # ---- online loss from gm slots ----
    # slot (k, cam, s) covers proxies perm[k][cam*512 + s*8 + (0..7)]
    V = gm.reshape(NCORES, B, C, NSLOT)                # [K, B, C, S]
    Vb = np.moveaxis(V, 1, 0).reshape(B, NCORES * C * NSLOT)  # [B, K*C*S]
    slot_cam = np.tile(np.repeat(np.arange(C), NSLOT), NCORES)
    slot_core = np.repeat(np.arange(NCORES), C * NSLOT)
    slot_off = np.tile(np.arange(NSLOT), NCORES * C)

    TGLOB = 96
    top_glob = np.argpartition(-Vb, TGLOB, axis=1)[:, :TGLOB]  # [B, 96]
    # per-camera top-4 slots (argmax coverage with fp16 slack)
    percam = []
    for c in range(C):
        cols = np.where(slot_cam == c)[0]
        sub = Vb[:, cols]
        a = np.argpartition(-sub, 4, axis=1)[:, :4]
        percam.append(cols[a])
    slots_sel = np.concatenate([top_glob] + percam, axis=1)    # [B, 96+32]
    nsel = slots_sel.shape[1]

    # candidate proxies: [B, nsel, G]
    base = (slot_core[slots_sel] * PSH + slot_cam[slots_sel] * PCAM
            + slot_off[slots_sel] * G)                         # [B, nsel]
    pid = perms.reshape(-1)[
        (slot_core[slots_sel] * PSH)[..., None]
        + (slot_cam[slots_sel] * PCAM + slot_off[slots_sel] * G)[..., None]
        + np.arange(G)[None, None, :]]                         # [B, nsel, G]
    pid_b = pid.reshape(B, nsel * G)
    cam_of_cand = np.repeat(slot_cam[slots_sel], G, axis=1)    # [B, nsel*G]

    memg = mem[pid_b]                                          # [B, J, D]
    s_cand = np.einsum("bd,bjd->bj", feat, memg)               # exact f32 score
    q_cand = np.einsum("bd,bjd->bj", memprx, memg)
    simsp = s_cand.astype(np.float64) + RATIO * q_cand.astype(np.float64)
    x_cand = INV_TEMP * s_cand.astype(np.float64)

    # dedupe duplicate proxies (same slot can appear in top_glob and percam)
    order_j = np.argsort(pid_b, axis=1)
    pid_srt = np.take_along_axis(pid_b, order_j, axis=1)
    dup = np.zeros_like(pid_b, dtype=bool)
    dup_srt = np.zeros_like(dup)
    dup_srt[:, 1:] = pid_srt[:, 1:] == pid_srt[:, :-1]
    np.put_along_axis(dup, order_j, dup_srt, axis=1)
    simsp = np.where(dup, -np.inf, simsp)

    # per-camera global argmax over candidates (exact values)
    tops_val = np.full((B, C), -np.inf)
    tops_j = np.zeros((B, C), np.int64)
    for c in range(C):
        m = cam_of_cand == c
        sub = np.where(m, simsp, -np.inf)
        a = sub.argmax(axis=1)
        tops_j[:, c] = a
        tops_val[:, c] = sub[rows, a]

    order = np.argsort(-tops_val, axis=1)[:, :POSK]            # [B, 3]
    chosen_j = np.take_along_axis(tops_j, order, axis=1)       # [B, 3]
    chosen_pid = np.take_along_axis(pid_b, chosen_j, axis=1)

    is_chosen = (pid_b[:, :, None] == chosen_pid[:, None, :]).any(axis=2)
    Vmask = np.where(is_chosen, -np.inf, simsp)
    sel_idx = np.argpartition(-Vmask, BG_KNN, axis=1)[:, :BG_KNN]

    x_chosen = np.take_along_axis(x_cand, chosen_j, axis=1)    # [B, 3]
    x_sel = np.take_along_axis(x_cand, sel_idx, axis=1)        # [B, 50]
    xA = np.concatenate([x_chosen, x_sel], axis=1)             # [B, 53]
    mA = xA.max(axis=1)
    lse3 = mA + np.log(np.exp(xA - mA[:, None]).sum(axis=1))
    online = lse3 - x_chosen.mean(axis=1)

    dbg = globals().get("_DEBUG_COMPS")
    if dbg is not None:
        dbg["intra"] = intra.copy()
        dbg["cross"] = cross.copy()
        dbg["online"] = online.copy()
    total = 0.0
    for c in range(C):
        m = cams_h == c
        if m.any():
            total += intra[m].mean() + cross[m].mean() + online[m].mean()
    return np.float32(total)


# revision 33
# speedup vs baseline: 1.0274x; 1.0041x over previous
"""CameraAwareMemory loss kernel for 8 Trainium2 NeuronCores.

Strategy: shard the P=32768 proxy bank over 8 cores (4096 proxies each,
columns permuted camera-major).  All matmuls run as fp8e4m3 DoubleRow
(2 k-planes per instruction, K=256 in one matmul, half-rate columns):
  score = f_hi.m_hi + f_hi.m_lo + f_lo.m_hi   (3 DR matmuls, hi/lo fp8
          splitting gives ~bf16 dot accuracy)
  sims' = g8.m_hi                             (1 DR matmul; selection only)
Per (camera, row-tile) 512-proxy PSUM cell:
  - sims cells: DVE tensor_reduce(max, X) computes the max of every
    8-column group in one pass (PSUM -> SBUF f16 [128, 64]); the group
    maxes ship to the host, which takes top slots (a provable superset
    of any top-k of the raw sims) and recomputes exact scores at the
    ~8x-expanded candidate list.
  - score cells: scalar-engine exp(x - mhat) with per-camera accumulate
    (device-exact partial softmax denominators), except N_V8 cells per
    row-tile which instead return their top-8 scores (DVE max8) so the
    host can reconstruct those cells' exp-sums from the dominant terms
    (tail of a 512-cell beyond rank 8 is ~e^-16 of the total).
The host merges: logsumexp from the per-camera exp-sums, intra/cross
losses from exact positive scores, online loss from the exact top-k
recomputation at gm candidates.
"""

import sys

import numpy as np

sys.path.insert(0, "/opt/trn_rl_repo")

import ml_dtypes

F8 = ml_dtypes.float8_e4m3          # IEEE e4m3 (max normal 240) == TRN FP8_EXP4

# ---- problem constants (hardcoded per spec) ----
P = 32768
D = 256
C = 8
B = 256
TEMP = 0.05
BG_KNN = 50
POSK = 3
BAL_W = 0.15
RATIO = (1.0 - BAL_W) / BAL_W        # sims' = score + RATIO*q (same order as sims)
INV_TEMP = 1.0 / TEMP                # 20.0
NCORES = 8
PSH = P // NCORES                    # 4096 proxies per core
PCAM = PSH // C                      # 512 proxies per (core, camera)
G = 8                                # group size for the sims group-max
NSLOT = PCAM // G                    # 64 slots per (core, cam) cell

S_F = 32.0                           # fp8 scale for feat
S_M = 512.0                          # fp8 scale for memT
S_G = 32.0                           # fp8 scale for g = feat + RATIO*mem[prx]
ACT_SCALE = INV_TEMP / (S_F * S_M)   # psum -> x conversion for exp

TABLE_PRELOAD = True                 # dummy Exp early to pay the table load
LHS_QUEUE = "sync"                   # which engine queue issues the lhs DMA
SIMS_PAIR = 1                        # sims cells per DVE reduce op (1/2)
EP_BUFS = 3
MINI_DELAY = 2
M_SHIFT = 112.5                      # global exp shift: > max_b 4.5*20*|feat_b|/16

# input DMA chunking of the bank (columns of the 4096-wide shard; each chunk
# carries both k-planes and both hi/lo halves in one dma_start)
CHUNKS = [0, 512, 1024, 1536, 2048, 2560, 3072, 3584, 4096]

_CACHE = {}


def _build_bass():
    import concourse.bacc as bacc
    import concourse.mybir as mybir
    import concourse.tile as tile
    from contextlib import ExitStack

    f32 = mybir.dt.float32
    f32r = mybir.dt.float32r
    bf16 = mybir.dt.bfloat16
    f8 = mybir.dt.float8e4
    AF = mybir.ActivationFunctionType
    ALU = mybir.AluOpType
    DR = mybir.MatmulPerfMode.DoubleRow

    nc = bacc.Bacc("TRN2", target_bir_lowering=False, debug=False)

    # lhs: [128, 2, 6*128] fp8 DR-packed: (f_hi rt0|rt1, f_lo rt0|rt1, g8 rt0|rt1)
    lhs_d = nc.dram_tensor("lhs", [128, 2 * 768], f8, kind="ExternalInput")
    # bank: [128, 2, 8192] fp8 DR-packed: m_hi (4096 cols) | m_lo (4096 cols)
    bank_d = nc.dram_tensor("bank", [128, 2 * 8192], f8, kind="ExternalInput")
    gm_d = nc.dram_tensor("gm", [B, C * NSLOT + C], bf16, kind="ExternalOutput")

    with tile.TileContext(nc) as tc, ExitStack() as ctx:
        consts = ctx.enter_context(tc.tile_pool(name="consts", bufs=1))
        ps_cam = ctx.enter_context(tc.tile_pool(name="ps_cam", bufs=2, space="PSUM"))
        ps_s = ctx.enter_context(tc.tile_pool(name="ps_s", bufs=1, space="PSUM"))
        ps_sum = ctx.enter_context(tc.tile_pool(name="ps_sum", bufs=1, space="PSUM"))
        small = ctx.enter_context(tc.tile_pool(name="small", bufs=2))
        epool = ctx.enter_context(tc.tile_pool(name="ep", bufs=EP_BUFS))
        outp = ctx.enter_context(tc.tile_pool(name="outp", bufs=2))

        lhs_sb = consts.tile([128, 2, 768], f8, tag="lhs")
        bank_sb = consts.tile([128, 2, 8192], f8, tag="bank")
        lhs_src = lhs_d.rearrange("p (two c) -> p two c", two=2)
        bank_src = bank_d.rearrange("p (two h c) -> p two h c", two=2, h=2)
        bank_dst = bank_sb.rearrange("p two (h c) -> p two h c", h=2)
        # g8 block first (feeds the sims pipeline), then the first bank chunk,
        # then the f blocks (score), then the remaining chunks
        nc.sync.dma_start(out=lhs_sb[:, :, 512:768], in_=lhs_src[:, :, 512:768])
        lo, hi = CHUNKS[0], CHUNKS[1]
        nc.sync.dma_start(
            out=bank_dst[:, :, :, lo:hi], in_=bank_src[:, :, :, lo:hi])
        nc.sync.dma_start(out=lhs_sb[:, :, 0:512], in_=lhs_src[:, :, 0:512])
        for g_ in range(1, len(CHUNKS) - 1):
            lo, hi = CHUNKS[g_], CHUNKS[g_ + 1]
            nc.sync.dma_start(
                out=bank_dst[:, :, :, lo:hi], in_=bank_src[:, :, :, lo:hi])

        # ones column for the partition-sum matmuls (f32r reads the f32 bits)
        ones_t = consts.tile([128, 1], bf16, tag="ones")
        nc.gpsimd.memset(ones_t[:], 1.0)
        mneg_t = consts.tile([128, 1], f32, tag="mneg")
        nc.gpsimd.memset(mneg_t[:], -float(M_SHIFT))
        if TABLE_PRELOAD:
            jnk = small.tile([128, 8], f32, tag="jnk")
            nc.gpsimd.memset(jnk[:], 0.0)
            tl_t = small.tile([128, 8], f32, tag="tl")
            nc.scalar.activation(tl_t[:], jnk[:], AF.Exp, bias=0.0, scale=0.0)

        # rhs views for the transposed score matmuls: all-256-batch fp8 blocks
        f_hi_all = lhs_sb[:, :, 0:256]
        f_lo_all = lhs_sb[:, :, 256:512]

        def g8_ap(rt):
            return lhs_sb[:, :, 512 + rt * 128:512 + (rt + 1) * 128]

        m_hi = bank_sb[:, :, 0:4096]
        m_lo = bank_sb[:, :, 4096:8192]

        gm_t = []
        for rt in range(2):
            gm_t.append(outp.tile([128, C * NSLOT + C], bf16, tag=f"gm{rt}",
                                  name=f"gm_{rt}"))
        _pair = {}
        st_all = ps_sum.tile([128, 2 * C], f32, tag="st", name="st_all")
        st = [st_all[:, ch * C:(ch + 1) * C] for ch in range(2)]

        ecam_of = {}

        def emit_minis(cam):
            ecam = ecam_of.pop(cam)
            for ch in range(2):
                for kb in range(4):
                    nc.tensor.matmul(
                        st[ch][:, cam:cam + 1],
                        lhsT=ecam[:, kb * 256 + ch * 128:
                                  kb * 256 + ch * 128 + 128],
                        rhs=ones_t[:],
                        start=(kb == 0), stop=(kb == 3))

        for cam in range(C):
            c0 = cam * 512
            # --- sims (normal orientation): per row-tile matmul + group-max
            for rt in range(2):
                if cam % SIMS_PAIR == 0 or (rt not in _pair):
                    pss = ps_s.tile([128, 512 * SIMS_PAIR], f32, tag=f"s{rt}",
                                    name=f"pss_{cam // SIMS_PAIR}_{rt}")
                    _pair[rt] = pss
                pss = _pair[rt]
                sl = (cam % SIMS_PAIR)
                nc.tensor.matmul(pss[:, sl * 512:(sl + 1) * 512],
                                 lhsT=g8_ap(rt), rhs=m_hi[:, :, c0:c0 + 512],
                                 start=True, stop=True, perf_mode=DR)
                if sl == SIMS_PAIR - 1:
                    nc.vector.tensor_reduce(
                        out=gm_t[rt][:, (cam + 1 - SIMS_PAIR) * NSLOT:
                                     (cam + 1) * NSLOT],
                        in_=pss.rearrange("p (q s g) -> p q s g",
                                          q=SIMS_PAIR, g=G),
                        axis=mybir.AxisListType.X, op=ALU.max)

            # --- transposed score: 4 k-blocks x 3 DR matmuls -> [128p, 1024b]
            camq = ps_cam.tile([128, 1024], f32, tag="cam", name=f"camq_{cam}")
            for kb in range(4):
                mh = m_hi[:, :, c0 + kb * 128:c0 + (kb + 1) * 128]
                ml = m_lo[:, :, c0 + kb * 128:c0 + (kb + 1) * 128]
                cols = slice(kb * 256, (kb + 1) * 256)
                nc.tensor.matmul(camq[:, cols], lhsT=mh, rhs=f_hi_all,
                                 start=True, stop=False, perf_mode=DR)
                nc.tensor.matmul(camq[:, cols], lhsT=ml, rhs=f_hi_all,
                                 start=False, stop=False, perf_mode=DR)
                nc.tensor.matmul(camq[:, cols], lhsT=mh, rhs=f_lo_all,
                                 start=False, stop=True, perf_mode=DR)
            # exp of the whole camera (global shift; f32 out for range)
            ecam = epool.tile([128, 1024], bf16, tag="e", name=f"e_{cam}")
            nc.scalar.activation(ecam[:], camq[:], AF.Exp,
                                 bias=mneg_t[:], scale=ACT_SCALE)
            ecam_of[cam] = ecam
            # software pipeline: sums for the PREVIOUS camera (its exp has had
            # a full camera's worth of PE work to complete)
            if cam >= MINI_DELAY:
                emit_minis(cam - MINI_DELAY)
            if cam == C - 1:
                for c_ in range(max(0, cam - MINI_DELAY + 1), cam + 1):
                    emit_minis(c_)
                for ch in range(2):
                    nc.scalar.activation(
                        gm_t[ch][:, C * NSLOT:], st[ch][:], AF.Copy,
                        bias=0.0, scale=1.0)
            if cam in (3, 5):
                lo = 0 if cam == 3 else 4 * NSLOT
                hi = (cam + 1) * NSLOT
                for rt in range(2):
                    r0, r1 = rt * 128, (rt + 1) * 128
                    nc.sync.dma_start(out=gm_d[r0:r1, lo:hi],
                                      in_=gm_t[rt][:, lo:hi])

        nf = 6 * NSLOT
        for rt in range(2):
            r0, r1 = rt * 128, (rt + 1) * 128
            nc.sync.dma_start(out=gm_d[r0:r1, nf:], in_=gm_t[rt][:, nf:])

    nc.compile()
    return nc


def _get_nc():
    if "nc" not in _CACHE:
        _CACHE["nc"] = _build_bass()
    return _CACHE["nc"]


def _run_device(in_maps, trace=False):
    from concourse.bass_utils import run_bass_kernel_spmd

    nc = _get_nc()
    res = run_bass_kernel_spmd(
        nc, in_maps, core_ids=list(range(NCORES)), trace=trace
    )
    return res


def _q8(x, clip=240.0):
    return np.clip(x, -clip, clip).astype(F8)


def _drpack(mat):
    """[256, X] (k-major) -> [128, 2, X]: plane j holds k = j*128 + p."""
    return np.stack([mat[0:128], mat[128:256]], axis=1)


def kernel(features, targets, cams, epoch, global_memory, all_pseudo_label,
           all_proxy_label, cam_proxies, label_proxies, _want_trace=False):
    feat = np.ascontiguousarray(np.asarray(features), dtype=np.float32)
    mem = np.ascontiguousarray(np.asarray(global_memory), dtype=np.float32)
    targets = np.asarray(targets).astype(np.int64)
    cams_h = np.asarray(cams).astype(np.int64)
    apl = np.asarray(all_proxy_label).astype(np.int64)
    apsl = np.asarray(all_pseudo_label).astype(np.int64)
    cam_prox = np.asarray(cam_proxies).astype(np.int64)
    lab_prox = np.asarray(label_proxies).astype(np.int64)

    prx = apl[targets]                      # [B] target proxy
    pseudo_y = apsl[targets]                # [B]
    pos_cols = lab_prox[pseudo_y]           # [B, K] positive proxies (cross)
    memprx = mem[prx]                       # [B, D]
    g = feat + np.float32(RATIO) * memprx   # [B, D]

    # camera of each proxy, and the camera-major permutation per core shard
    cam_of_p = np.empty(P, np.int64)
    cam_of_p[cam_prox.reshape(-1)] = np.repeat(np.arange(C), cam_prox.shape[1])
    perms = np.empty((NCORES, PSH), np.int64)
    for k in range(NCORES):
        ids = np.arange(k * PSH, (k + 1) * PSH)
        parts = [ids[cam_of_p[ids] == c] for c in range(C)]
        assert all(len(p) == PCAM for p in parts), "camera layout mismatch"
        perms[k] = np.concatenate(parts)

    # fp8 hi/lo quantization (shared across cores; shards sliced per core)
    featT = feat.T                                   # [D, B]
    f_hi8 = _q8(featT * np.float32(S_F))
    f_lo8 = _q8(featT * np.float32(S_F) - f_hi8.astype(np.float32))
    g_hi8 = _q8(g.T * np.float32(S_G))
    memT = mem.T                                     # [D, P]
    assert np.abs(featT).max() * S_F < 239, "feat fp8 scale overflow"
    assert np.abs(g).max() * S_G < 239, "g fp8 scale overflow"
    assert np.abs(memT).max() * S_M < 239, "mem fp8 scale overflow"

    in_maps = []
    for k in range(NCORES):
        mT = memT[:, perms[k]] * np.float32(S_M)     # [D, 4096]
        m_hi8 = _q8(mT)
        m_lo8 = _q8(mT - m_hi8.astype(np.float32))
        lhs = np.concatenate([
            _drpack(f_hi8[:, 0:128]), _drpack(f_hi8[:, 128:256]),
            _drpack(f_lo8[:, 0:128]), _drpack(f_lo8[:, 128:256]),
            _drpack(g_hi8[:, 0:128]), _drpack(g_hi8[:, 128:256]),
        ], axis=2)                                   # [128, 2, 768]
        bank = np.concatenate([_drpack(m_hi8), _drpack(m_lo8)], axis=2)
        in_maps.append({
            "lhs": np.ascontiguousarray(lhs.reshape(128, -1)),
            "bank": np.ascontiguousarray(bank.reshape(128, -1)),
        })

    res = _run_device(in_maps, trace=_want_trace)
    results = res.results
    if _want_trace:
        _CACHE["last_exec_time_ns"] = res.exec_time_ns

    gmall = np.stack([r["gm"] for r in results]).astype(np.float32)   # [K,B,C*64+C]
    gm = gmall[:, :, 0:C * NSLOT]
    parts = gmall[:, :, C * NSLOT:].astype(np.float64)                 # [K,B,C]

    rows = np.arange(B)
    Zc = parts.sum(axis=0)                             # [B, C]
    lse_full = M_SHIFT + np.log(Zc.sum(axis=1))
    lse_cam = M_SHIFT + np.log(Zc[rows, cams_h])

    x_prx = INV_TEMP * np.einsum("bd,bd->b", feat.astype(np.float64),
                                 memprx.astype(np.float64))
    present = cam_of_p[prx] == cams_h
    intra = np.where(present, lse_cam - x_prx, 0.0)

    x_pos = INV_TEMP * np.einsum("bd,bkd->bk", feat.astype(np.float64),
                                 mem[pos_cols].astype(np.float64))
    cross = lse_full - x_pos.mean(axis=1)

    # ---- online loss from gm slots ----
    # slot (k, cam, s) covers proxies perm[k][cam*512 + s*8 + (0..7)]
    V = gm.reshape(NCORES, B, C, NSLOT)                # [K, B, C, S]
    Vb = np.moveaxis(V, 1, 0).reshape(B, NCORES * C * NSLOT)  # [B, K*C*S]
    slot_cam = np.tile(np.repeat(np.arange(C), NSLOT), NCORES)
    slot_core = np.repeat(np.arange(NCORES), C * NSLOT)
    slot_off = np.tile(np.arange(NSLOT), NCORES * C)

    TGLOB = 96
    top_glob = np.argpartition(-Vb, TGLOB, axis=1)[:, :TGLOB]  # [B, 96]
    # per-camera top-4 slots (argmax coverage with fp16 slack)
    percam = []
    for c in range(C):
        cols = np.where(slot_cam == c)[0]
        sub = Vb[:, cols]
        a = np.argpartition(-sub, 4, axis=1)[:, :4]
        percam.append(cols[a])
    slots_sel = np.concatenate([top_glob] + percam, axis=1)    # [B, 96+32]
    nsel = slots_sel.shape[1]

    # candidate proxies: [B, nsel, G]
    base = (slot_core[slots_sel] * PSH + slot_cam[slots_sel] * PCAM
            + slot_off[slots_sel] * G)                         # [B, nsel]
    pid = perms.reshape(-1)[
        (slot_core[slots_sel] * PSH)[..., None]
        + (slot_cam[slots_sel] * PCAM + slot_off[slots_sel] * G)[..., None]
        + np.arange(G)[None, None, :]]                         # [B, nsel, G]
    pid_b = pid.reshape(B, nsel * G)
    cam_of_cand = np.repeat(slot_cam[slots_sel], G, axis=1)    # [B, nsel*G]

    memg = mem[pid_b]                                          # [B, J, D]
    s_cand = np.einsum("bd,bjd->bj", feat, memg)               # exact f32 score
    q_cand = np.einsum("bd,bjd->bj", memprx, memg)
    simsp = s_cand.astype(np.float64) + RATIO * q_cand.astype(np.float64)
    x_cand = INV_TEMP * s_cand.astype(np.float64)

    # dedupe duplicate proxies (same slot can appear in top_glob and percam)
    order_j = np.argsort(pid_b, axis=1)
    pid_srt = np.take_along_axis(pid_b, order_j, axis=1)
    dup = np.zeros_like(pid_b, dtype=bool)
    dup_srt = np.zeros_like(dup)
    dup_srt[:, 1:] = pid_srt[:, 1:] == pid_srt[:, :-1]
    np.put_along_axis(dup, order_j, dup_srt, axis=1)
    simsp = np.where(dup, -np.inf, simsp)

    # per-camera global argmax over candidates (exact values)
    tops_val = np.full((B, C), -np.inf)
    tops_j = np.zeros((B, C), np.int64)
    for c in range(C):
        m = cam_of_cand == c
        sub = np.where(m, simsp, -np.inf)
        a = sub.argmax(axis=1)
        tops_j[:, c] = a
        tops_val[:, c] = sub[rows, a]

    order = np.argsort(-tops_val, axis=1)[:, :POSK]            # [B, 3]
    chosen_j = np.take_along_axis(tops_j, order, axis=1)       # [B, 3]
    chosen_pid = np.take_along_axis(pid_b, chosen_j, axis=1)

    is_chosen = (pid_b[:, :, None] == chosen_pid[:, None, :]).any(axis=2)
    Vmask = np.where(is_chosen, -np.inf, simsp)
    sel_idx = np.argpartition(-Vmask, BG_KNN, axis=1)[:, :BG_KNN]

    x_chosen = np.take_along_axis(x_cand, chosen_j, axis=1)    # [B, 3]
    x_sel = np.take_along_axis(x_cand, sel_idx, axis=1)        # [B, 50]
    xA = np.concatenate([x_chosen, x_sel], axis=1)             # [B, 53]
    mA = xA.max(axis=1)
    lse3 = mA + np.log(np.exp(xA - mA[:, None]).sum(axis=1))
    online = lse3 - x_chosen.mean(axis=1)

    dbg = globals().get("_DEBUG_COMPS")
    if dbg is not None:
        dbg["intra"] = intra.copy()
        dbg["cross"] = cross.copy()
        dbg["online"] = online.copy()
    total = 0.0
    for c in range(C):
        m = cams_h == c
        if m.any():
            total += intra[m].mean() + cross[m].mean() + online[m].mean()
    return np.float32(total)
